# revision 23
# baseline (speedup 1.0000x reference)
"""Trainium2 Bass kernel for nn_CogitatDeepSetNorm (segment_reduce, 8 cores).

Math: the reference network collapses to a rank-1 structure --
  rowsum_i = sum_d x[i, d]                                     (per row)
  segsum_s = sum_{i: sub_i = s} rowsum_i ; count_s = |{i: sub_i = s}|
  s_val_s  = relu(Gamma * segsum_s / count_s)                  (per segment)
  out[i, :] = relu(Lambda * rowsum_i + 128 * Lambda * s_val_{sub_i})
so the kernel only has to stream x once (128 MiB read), do a 64-bin segment
reduce of the rowsums, and write the rank-1 output (128 MiB write): purely
memory-bound.

Distribution: data-parallel over rows, 1/8 of the rows per NeuronCore.  The
cross-core combine of the per-core [segsum | count] vectors (8 x 512 B) is
done on the host between two NEFF launches ("kernel fission") -- a
host-mediated all-reduce.  An on-device AllGather version (see
kernel_single.py) was measured at ~30 us of ncfw/mesh latency per launch;
the second NEFF's fixed overhead (~20 us) is cheaper and also removes the
collective-entry barrier.

Per-core layout: local row r -> (partition p = r // 128, group f = r % 128),
chosen so every x/out DMA moves 8 KiB contiguous per partition.  (Fat
descriptors matter: 1 KiB-row access patterns cost ~10 ns/descriptor of
HWDGE sequencer time and throttle the whole kernel.)

Kernel A (one pass over x, DMA-bound ~64 us):
  - 1 MiB x tiles -> DVE tensor_reduce rowsums (f32)
  - segment reduce via PE matmuls: psum[2, 64] += [rs_f, 1]^T @ onehot_f
    with bf16 operands (one-hots are exact in bf16; rowsum rounding to bf16
    only affects the segment-mean path, where the output sensitivity is
    ~1e-4; the precision-critical Lambda*rowsum bias stays f32)
  - outputs: rowsums [128, 128] f32 + partial [segsum | count] [2, 64]

Host: sums the 8 partial vectors, forms s_val (1 KB of math), pre-scales.

Kernel C (one pass over out, DMA-bound ~64 us):
  - gathers s_val per row with a bf16 one-hot-transpose matmul whose
    broadcast rhs fans the value across all 256 output columns
    (psum[128, 256] = ohT_f^T @ svalb)
  - one relu-with-per-partition-bias op per group (ACT and DVE alternate)
  - 1 MiB output store per chunk

The bf16 one-hot encodings of `sub` are prepared on the host in both
orientations (rows-on-partitions [128, 128, 64] for kernel A,
segments-on-partitions [64, 16384] for kernel C); they replace the raw
index input at +4 MiB DMA per core and keep the vector engine free.

The one-hots are stored in fp8e4 (0/1 exact; mixed fp8 lhsT x bf16 rhs
matmul verified on HW), halving their DMA cost.

Measured on trn2 (8 cores): ~62 us + ~63 us = 124-132 us total HW exec
(was 241 us for the first working version), scale-relative absmax error
~7e-7 vs the fp32 reference.
"""

import sys

if "/opt/trn_rl_repo" not in sys.path:
    sys.path.insert(0, "/opt/trn_rl_repo")

import numpy as np

N = 131072
D = 256
S = 64          # n_subs
MID = 128      # middle dims
N_CORES = 8
NL = N // N_CORES          # rows per core = 16384
P = 128                    # partitions
F = NL // P                # row-groups per core = 128
CH = 8                     # row-groups per chunk (1 MiB tiles)
NCHUNK = F // CH           # 16

TRACE = False              # test harness sets this for profiling
LAST_RESULT = None         # (resA, resC) of the last run

_build_cache = {}


def _build():
    from contextlib import ExitStack

    import concourse.bacc as bacc
    import concourse.tile as tile
    from concourse import mybir

    f32 = mybir.dt.float32
    bf16 = mybir.dt.bfloat16
    fp8 = mybir.dt.float8e4
    Alu = mybir.AluOpType
    Act = mybir.ActivationFunctionType
    X = mybir.AxisListType.X

    # ---------------- kernel A: rowsums + per-core segment partials --------
    ncA = bacc.Bacc("TRN2", target_bir_lowering=False, debug=False,
                    enable_asserts=False, num_devices=N_CORES)
    x_d = ncA.dram_tensor("x", [NL, D], f32, kind="ExternalInput").ap()
    oh_d = ncA.dram_tensor("oh", [P, F, S], fp8, kind="ExternalInput").ap()
    rs_out_d = ncA.dram_tensor("rs", [P, F], f32, kind="ExternalOutput").ap()
    seg_out_d = ncA.dram_tensor("seg", [S, 2], f32, kind="ExternalOutput").ap()
    x_v = x_d.rearrange("(p f) d -> p f d", p=P)

    with tile.TileContext(ncA) as tc, ExitStack() as ctx:
        nc = ncA
        singles = ctx.enter_context(tc.tile_pool(name="singles", bufs=1))
        xpool = ctx.enter_context(tc.tile_pool(name="xpool", bufs=6))
        psa = ctx.enter_context(tc.tile_pool(name="psa", bufs=1, space="PSUM"))

        oh_all = singles.tile([P, F, S], fp8)
        for q in range(4):
            nc.scalar.dma_start(out=oh_all[:, q * F // 4:(q + 1) * F // 4, :],
                                in_=oh_d[:, q * F // 4:(q + 1) * F // 4, :])
        rs_f32 = singles.tile([P, F], f32)
        rs2b = singles.tile([P, 2 * F], bf16)    # interleaved [rs, 1] bf16
        nc.vector.memset(rs2b[:, 1:2 * F:2], 1.0)
        psum_seg = psa.tile([S, 2], f32)

        for n in range(NCHUNK):
            xt = xpool.tile([P, CH, D], f32)
            nc.sync.dma_start(out=xt, in_=x_v[:, CH * n: CH * (n + 1), :])
            nc.vector.tensor_reduce(
                out=rs_f32[:, CH * n: CH * (n + 1)], in_=xt,
                axis=X, op=Alu.add)
            nc.vector.tensor_copy(
                rs2b[:, 2 * CH * n: 2 * CH * (n + 1): 2],
                rs_f32[:, CH * n: CH * (n + 1)])
            for a in range(CH):
                f = CH * n + a
                nc.tensor.matmul(
                    psum_seg, lhsT=oh_all[:, f, :],
                    rhs=rs2b[:, 2 * f:2 * f + 2],
                    start=(f == 0), stop=(f == F - 1))

        seg_sb = singles.tile([S, 2], f32)
        nc.scalar.copy(seg_sb, psum_seg)        # ACT: DVE busy with reduces
        nc.sync.dma_start(out=seg_out_d, in_=seg_sb)
        for q in range(4):
            nc.scalar.dma_start(out=rs_out_d[:, q * F // 4:(q + 1) * F // 4],
                                in_=rs_f32[:, q * F // 4:(q + 1) * F // 4])
    ncA.compile()

    # ---------------- kernel C: gather + relu + broadcast store ------------
    ncC = bacc.Bacc("TRN2", target_bir_lowering=False, debug=False,
                    enable_asserts=False, num_devices=N_CORES)
    ohT_d = ncC.dram_tensor("ohT", [S, NL], fp8, kind="ExternalInput").ap()
    rs_in_d = ncC.dram_tensor("rss", [P, F], f32, kind="ExternalInput").ap()
    svb_d = ncC.dram_tensor("svb", [S, D], bf16, kind="ExternalInput").ap()
    out_d = ncC.dram_tensor("out", [NL, D], f32, kind="ExternalOutput").ap()
    out_v = out_d.rearrange("(p f) d -> p f d", p=P)

    with tile.TileContext(ncC) as tc, ExitStack() as ctx:
        nc = ncC
        singles = ctx.enter_context(tc.tile_pool(name="singles", bufs=1))
        outpool = ctx.enter_context(tc.tile_pool(name="outpool", bufs=6))
        psc = ctx.enter_context(tc.tile_pool(name="psc", bufs=8, space="PSUM"))

        onehotT = singles.tile([S, NL], fp8)
        for q in range(4):
            nc.scalar.dma_start(out=onehotT[:, q * NL // 4:(q + 1) * NL // 4],
                                in_=ohT_d[:, q * NL // 4:(q + 1) * NL // 4])
        rs_scaled = singles.tile([P, F], f32)    # host pre-scales by Lambda
        nc.sync.dma_start(out=rs_scaled, in_=rs_in_d)
        sval_b = singles.tile([S, D], bf16)      # host-broadcast 128*Lam*sval
        nc.sync.dma_start(out=sval_b, in_=svb_d)

        H = CH // 2
        for n in range(NCHUNK):
            # each engine owns one contiguous half-chunk tile, so ACT and DVE
            # writers never interleave on the same tile (no cross-engine
            # serialization), and each half stores as soon as it is done
            ot_a = outpool.tile([P, H, D], f32, tag="ota")
            ot_b = outpool.tile([P, H, D], f32, tag="otb")
            for a in range(CH):
                f = CH * n + a
                pc = psc.tile([P, D], f32)
                nc.tensor.matmul(
                    pc, lhsT=onehotT[:, P * f:P * (f + 1)],
                    rhs=sval_b, start=True, stop=True)
                if a < H:
                    nc.scalar.activation(
                        out=ot_a[:, a, :], in_=pc, func=Act.Relu,
                        bias=rs_scaled[:, f:f + 1], scale=1.0)
                else:
                    nc.vector.tensor_scalar(
                        out=ot_b[:, a - H, :], in0=pc,
                        scalar1=rs_scaled[:, f:f + 1], scalar2=0.0,
                        op0=Alu.add, op1=Alu.max)
            nc.sync.dma_start(out=out_v[:, CH * n: CH * n + H, :], in_=ot_a)
            nc.sync.dma_start(out=out_v[:, CH * n + H: CH * (n + 1), :],
                              in_=ot_b)
    ncC.compile()
    return ncA, ncC


def _ensure_ntff_hook_module():
    # bass_utils imports antenv.axon_hooks when tracing is requested (e.g.
    # via a BASS_TRACE env); this image's antenv lacks it.  Register a stub
    # (get -> None makes bass_utils skip tracing gracefully) unless a real
    # hook module was already installed by the test harness.
    import types
    if "antenv.axon_hooks" in sys.modules:
        return
    try:
        import antenv
        import antenv.axon_hooks  # noqa: F401
    except ImportError:
        mod = types.ModuleType("antenv.axon_hooks")
        _state = {"hook": None}
        mod.set_axon_ntff_profile_hook = lambda h: _state.__setitem__("hook", h)
        mod.get_axon_ntff_profile_hook = lambda: _state["hook"]
        sys.modules["antenv.axon_hooks"] = mod
        antenv.axon_hooks = mod


def kernel(x, sub, Gamma, Lambda):
    import ml_dtypes
    from concourse import bass_utils

    _ensure_ntff_hook_module()

    global LAST_RESULT
    x = np.ascontiguousarray(np.asarray(x, dtype=np.float32))
    sub = np.asarray(sub).astype(np.int32)
    gamma = float(np.asarray(Gamma).reshape(-1)[0])
    lam = float(np.asarray(Lambda).reshape(-1)[0])

    # Gamma/Lambda are applied on the host side of the fission boundary, so
    # the compiled NEFFs are parameter-independent.
    if "k" not in _build_cache:
        _build_cache["k"] = _build()
    ncA, ncC = _build_cache["k"]

    seg_ids = np.arange(S, dtype=np.int32)
    sub_pf = [sub[c * NL:(c + 1) * NL].reshape(P, F) for c in range(N_CORES)]
    in_maps_a = []
    for c in range(N_CORES):
        oh = (sub_pf[c][:, :, None] == seg_ids).astype(ml_dtypes.float8_e4m3)
        in_maps_a.append({
            "x": x[c * NL:(c + 1) * NL],
            "oh": np.ascontiguousarray(oh),
        })

    resA = bass_utils.run_bass_kernel_spmd(
        ncA, in_maps_a, core_ids=list(range(N_CORES)), trace=TRACE)

    # host: combine the 8 partial [segsum | count] vectors -> s_val
    # (1 KB of math; empty segments fall back to s_val = 0, matching the
    # on-device max(count, 1) guard)
    seg = sum(resA.results[c]["seg"].astype(np.float64)
              for c in range(N_CORES))
    segsum, counts = seg[:, 0], seg[:, 1]
    means = segsum / np.maximum(counts, 1.0)
    sval = np.maximum(gamma * means, 0.0) * (MID * lam)
    svb = np.ascontiguousarray(
        np.broadcast_to(sval.astype(np.float32)[:, None],
                        (S, D)).astype(ml_dtypes.bfloat16))

    in_maps_c = []
    for c in range(N_CORES):
        ohT = (seg_ids[:, None] == sub_pf[c].T.reshape(1, -1)).astype(
            ml_dtypes.float8_e4m3)
        in_maps_c.append({
            "ohT": np.ascontiguousarray(ohT),
            "rss": resA.results[c]["rs"] * np.float32(lam),
            "svb": svb,
        })

    resC = bass_utils.run_bass_kernel_spmd(
        ncC, in_maps_c, core_ids=list(range(N_CORES)), trace=TRACE)
    LAST_RESULT = (resA, resC)

    out = np.empty((N, D), dtype=np.float32)
    for c in range(N_CORES):
        out[c * NL:(c + 1) * NL] = resC.results[c]["out"]
    return out


# revision 24
# speedup vs baseline: 1.0195x; 1.0195x over previous
"""Trainium2 Bass kernel for nn_CogitatDeepSetNorm (segment_reduce, 8 cores).

Math: the reference network collapses to a rank-1 structure --
  rowsum_i = sum_d x[i, d]                                     (per row)
  segsum_s = sum_{i: sub_i = s} rowsum_i ; count_s = |{i: sub_i = s}|
  s_val_s  = relu(Gamma * segsum_s / count_s)                  (per segment)
  out[i, :] = relu(Lambda * rowsum_i + 128 * Lambda * s_val_{sub_i})
so the kernel only has to stream x once (128 MiB read), do a 64-bin segment
reduce of the rowsums, and write the rank-1 output (128 MiB write): purely
memory-bound.

Distribution: data-parallel over rows, 1/8 of the rows per NeuronCore.  The
cross-core combine of the per-core [segsum | count] vectors (8 x 512 B) is
done on the host between two NEFF launches ("kernel fission") -- a
host-mediated all-reduce.  An on-device AllGather version (see
kernel_single.py) was measured at ~30 us of ncfw/mesh latency per launch;
the second NEFF's fixed overhead (~20 us) is cheaper and also removes the
collective-entry barrier.

Per-core layout: local row r -> (partition p = r // 128, group f = r % 128),
chosen so every x/out DMA moves 8 KiB contiguous per partition.  (Fat
descriptors matter: 1 KiB-row access patterns cost ~10 ns/descriptor of
HWDGE sequencer time and throttle the whole kernel.)

Kernel A (one pass over x, DMA-bound ~64 us):
  - 1 MiB x tiles -> DVE tensor_reduce rowsums (f32)
  - segment reduce via PE matmuls: psum[2, 64] += [rs_f, 1]^T @ onehot_f
    with bf16 operands (one-hots are exact in bf16; rowsum rounding to bf16
    only affects the segment-mean path, where the output sensitivity is
    ~1e-4; the precision-critical Lambda*rowsum bias stays f32)
  - outputs: rowsums [128, 128] f32 + partial [segsum | count] [2, 64]

Host: sums the 8 partial vectors, forms s_val (1 KB of math), pre-scales.

Kernel C (one pass over out, DMA-bound ~64 us):
  - gathers s_val per row with a bf16 one-hot-transpose matmul whose
    broadcast rhs fans the value across all 256 output columns
    (psum[128, 256] = ohT_f^T @ svalb)
  - one relu-with-per-partition-bias op per group (ACT and DVE alternate)
  - 1 MiB output store per chunk

The bf16 one-hot encodings of `sub` are prepared on the host in both
orientations (rows-on-partitions [128, 128, 64] for kernel A,
segments-on-partitions [64, 16384] for kernel C); they replace the raw
index input at +4 MiB DMA per core and keep the vector engine free.

The one-hots are stored in fp8e4 (0/1 exact; mixed fp8 lhsT x bf16 rhs
matmul verified on HW), halving their DMA cost.

Measured on trn2 (8 cores): ~62 us + ~63 us = 124-132 us total HW exec
(was 241 us for the first working version), scale-relative absmax error
~7e-7 vs the fp32 reference.
"""

import sys

if "/opt/trn_rl_repo" not in sys.path:
    sys.path.insert(0, "/opt/trn_rl_repo")

import numpy as np

N = 131072
D = 256
S = 64          # n_subs
MID = 128      # middle dims
N_CORES = 8
NL = N // N_CORES          # rows per core = 16384
P = 128                    # partitions
F = NL // P                # row-groups per core = 128
CH = 8                     # row-groups per chunk (1 MiB tiles)
NCHUNK = F // CH           # 16

TRACE = False              # test harness sets this for profiling
LAST_RESULT = None         # (resA, resC) of the last run

_build_cache = {}


def _build():
    from contextlib import ExitStack

    import concourse.bacc as bacc
    import concourse.tile as tile
    from concourse import mybir

    f32 = mybir.dt.float32
    bf16 = mybir.dt.bfloat16
    fp8 = mybir.dt.float8e4
    Alu = mybir.AluOpType
    Act = mybir.ActivationFunctionType
    X = mybir.AxisListType.X

    # ---------------- kernel A: rowsums + per-core segment partials --------
    ncA = bacc.Bacc("TRN2", target_bir_lowering=False, debug=False,
                    enable_asserts=False, num_devices=N_CORES)
    x_d = ncA.dram_tensor("x", [NL, D], f32, kind="ExternalInput").ap()
    oh_d = ncA.dram_tensor("oh", [P, F, S], fp8, kind="ExternalInput").ap()
    rs_out_d = ncA.dram_tensor("rs", [P, F], f32, kind="ExternalOutput").ap()
    seg_out_d = ncA.dram_tensor("seg", [S, 2], f32, kind="ExternalOutput").ap()
    x_v = x_d.rearrange("(p f) d -> p f d", p=P)

    with tile.TileContext(ncA) as tc, ExitStack() as ctx:
        nc = ncA
        singles = ctx.enter_context(tc.tile_pool(name="singles", bufs=1))
        xpool = ctx.enter_context(tc.tile_pool(name="xpool", bufs=6))
        psa = ctx.enter_context(tc.tile_pool(name="psa", bufs=1, space="PSUM"))

        oh_all = singles.tile([P, F, S], fp8)
        for q in range(4):
            nc.scalar.dma_start(out=oh_all[:, q * F // 4:(q + 1) * F // 4, :],
                                in_=oh_d[:, q * F // 4:(q + 1) * F // 4, :])
        rs_f32 = singles.tile([P, F], f32)
        rs2b = singles.tile([P, 2 * F], bf16)    # interleaved [rs, 1] bf16
        nc.vector.memset(rs2b[:, 1:2 * F:2], 1.0)
        psum_seg = psa.tile([S, 2], f32)

        for n in range(NCHUNK):
            xt = xpool.tile([P, CH, D], f32)
            nc.sync.dma_start(out=xt, in_=x_v[:, CH * n: CH * (n + 1), :])
            nc.vector.tensor_reduce(
                out=rs_f32[:, CH * n: CH * (n + 1)], in_=xt,
                axis=X, op=Alu.add)
            nc.vector.tensor_copy(
                rs2b[:, 2 * CH * n: 2 * CH * (n + 1): 2],
                rs_f32[:, CH * n: CH * (n + 1)])
            for a in range(CH):
                f = CH * n + a
                nc.tensor.matmul(
                    psum_seg, lhsT=oh_all[:, f, :],
                    rhs=rs2b[:, 2 * f:2 * f + 2],
                    start=(f == 0), stop=(f == F - 1))

        seg_sb = singles.tile([S, 2], f32)
        nc.scalar.copy(seg_sb, psum_seg)        # ACT: DVE busy with reduces
        nc.sync.dma_start(out=seg_out_d, in_=seg_sb)
        for q in range(4):
            nc.scalar.dma_start(out=rs_out_d[:, q * F // 4:(q + 1) * F // 4],
                                in_=rs_f32[:, q * F // 4:(q + 1) * F // 4])
    ncA.compile()

    # ---------------- kernel C: gather + relu + broadcast store ------------
    ncC = bacc.Bacc("TRN2", target_bir_lowering=False, debug=False,
                    enable_asserts=False, num_devices=N_CORES)
    ohT_d = ncC.dram_tensor("ohT", [S, NL], fp8, kind="ExternalInput").ap()
    rs_in_d = ncC.dram_tensor("rss", [P, F], f32, kind="ExternalInput").ap()
    svb_d = ncC.dram_tensor("svb", [S, D], bf16, kind="ExternalInput").ap()
    out_d = ncC.dram_tensor("out", [NL, D], f32, kind="ExternalOutput").ap()
    out_v = out_d.rearrange("(p f) d -> p f d", p=P)

    with tile.TileContext(ncC) as tc, ExitStack() as ctx:
        nc = ncC
        singles = ctx.enter_context(tc.tile_pool(name="singles", bufs=1))
        outpool = ctx.enter_context(tc.tile_pool(name="outpool", bufs=6))
        psc = ctx.enter_context(tc.tile_pool(name="psc", bufs=8, space="PSUM"))

        onehotT = singles.tile([S, NL], fp8)
        for q in range(4):
            nc.scalar.dma_start(out=onehotT[:, q * NL // 4:(q + 1) * NL // 4],
                                in_=ohT_d[:, q * NL // 4:(q + 1) * NL // 4])
        rs_scaled = singles.tile([P, F], f32)    # host pre-scales by Lambda
        nc.sync.dma_start(out=rs_scaled, in_=rs_in_d)
        sval_b = singles.tile([S, D], bf16)      # host-broadcast 128*Lam*sval
        nc.sync.dma_start(out=sval_b, in_=svb_d)

        for n in range(NCHUNK):
            ot = outpool.tile([P, CH, D], f32)
            for a in range(CH):
                f = CH * n + a
                pc = psc.tile([P, D], f32)
                nc.tensor.matmul(
                    pc, lhsT=onehotT[:, P * f:P * (f + 1)],
                    rhs=sval_b, start=True, stop=True)
                if f % 2 == 0:
                    nc.scalar.activation(
                        out=ot[:, a, :], in_=pc, func=Act.Relu,
                        bias=rs_scaled[:, f:f + 1], scale=1.0)
                else:
                    nc.vector.tensor_scalar(
                        out=ot[:, a, :], in0=pc,
                        scalar1=rs_scaled[:, f:f + 1], scalar2=0.0,
                        op0=Alu.add, op1=Alu.max)
            nc.sync.dma_start(out=out_v[:, CH * n: CH * (n + 1), :], in_=ot)
    ncC.compile()
    return ncA, ncC


def _ensure_ntff_hook_module():
    # bass_utils imports antenv.axon_hooks when tracing is requested (e.g.
    # via a BASS_TRACE env); this image's antenv lacks it.  Register a stub
    # (get -> None makes bass_utils skip tracing gracefully) unless a real
    # hook module was already installed by the test harness.
    import types
    if "antenv.axon_hooks" in sys.modules:
        return
    try:
        import antenv
        import antenv.axon_hooks  # noqa: F401
    except ImportError:
        mod = types.ModuleType("antenv.axon_hooks")
        _state = {"hook": None}
        mod.set_axon_ntff_profile_hook = lambda h: _state.__setitem__("hook", h)
        mod.get_axon_ntff_profile_hook = lambda: _state["hook"]
        sys.modules["antenv.axon_hooks"] = mod
        antenv.axon_hooks = mod


def kernel(x, sub, Gamma, Lambda):
    import ml_dtypes
    from concourse import bass_utils

    _ensure_ntff_hook_module()

    global LAST_RESULT
    x = np.ascontiguousarray(np.asarray(x, dtype=np.float32))
    sub = np.asarray(sub).astype(np.int32)
    gamma = float(np.asarray(Gamma).reshape(-1)[0])
    lam = float(np.asarray(Lambda).reshape(-1)[0])

    # Gamma/Lambda are applied on the host side of the fission boundary, so
    # the compiled NEFFs are parameter-independent.
    if "k" not in _build_cache:
        _build_cache["k"] = _build()
    ncA, ncC = _build_cache["k"]

    seg_ids = np.arange(S, dtype=np.int32)
    sub_pf = [sub[c * NL:(c + 1) * NL].reshape(P, F) for c in range(N_CORES)]
    in_maps_a = []
    for c in range(N_CORES):
        oh = (sub_pf[c][:, :, None] == seg_ids).astype(ml_dtypes.float8_e4m3)
        in_maps_a.append({
            "x": x[c * NL:(c + 1) * NL],
            "oh": np.ascontiguousarray(oh),
        })

    resA = bass_utils.run_bass_kernel_spmd(
        ncA, in_maps_a, core_ids=list(range(N_CORES)), trace=TRACE)

    # host: combine the 8 partial [segsum | count] vectors -> s_val
    # (1 KB of math; empty segments fall back to s_val = 0, matching the
    # on-device max(count, 1) guard)
    seg = sum(resA.results[c]["seg"].astype(np.float64)
              for c in range(N_CORES))
    segsum, counts = seg[:, 0], seg[:, 1]
    means = segsum / np.maximum(counts, 1.0)
    sval = np.maximum(gamma * means, 0.0) * (MID * lam)
    svb = np.ascontiguousarray(
        np.broadcast_to(sval.astype(np.float32)[:, None],
                        (S, D)).astype(ml_dtypes.bfloat16))

    in_maps_c = []
    for c in range(N_CORES):
        ohT = (seg_ids[:, None] == sub_pf[c].T.reshape(1, -1)).astype(
            ml_dtypes.float8_e4m3)
        in_maps_c.append({
            "ohT": np.ascontiguousarray(ohT),
            "rss": resA.results[c]["rs"] * np.float32(lam),
            "svb": svb,
        })

    resC = bass_utils.run_bass_kernel_spmd(
        ncC, in_maps_c, core_ids=list(range(N_CORES)), trace=TRACE)
    LAST_RESULT = (resA, resC)

    out = np.empty((N, D), dtype=np.float32)
    for c in range(N_CORES):
        out[c * NL:(c + 1) * NL] = resC.results[c]["out"]
    return out
